# revision 22
# baseline (speedup 1.0000x reference)
"""Confusion-matrix (joint histogram) kernel for Trainium2.

Math: out[b, i, j] = #{pixels p in batch b : yp[b,p] == i and y[b,p] == j}
for i, j in [0, 21). Inputs yp, y are [8, 2048, 2048] int32, values in [0, 21).

Strategy (per NeuronCore, core c processes batch c):
  - THERMOMETER encoding: planes[level i] = (x >= i) instead of one-hots.
    TensorE then accumulates TH[i, j] = #{yp >= i and y >= j}; the host
    recovers counts via an exact 2-D finite difference (integer math in
    float64). Thermometer levels are cheaper to generate than one-hots:
      level 0  = all-ones const plane (memset once per buffer, never redone)
      level 1  = Sign(x) on ScalarE (1 op)
      levels 18, 19 = sigmoid(100*(x - i + 0.5)) on ScalarE (1 op each;
        saturates to exactly {0, 1} in bf16 away from the knee, residual
        mask error <= 2e-22 per pixel -- far below count integrality)
      level 20 = Relu(2x - 39) on ScalarE (1 op)
      levels 2..17 = tensor_scalar(is_ge) on VectorE (16 ops, 4x perf mode)
  - layout: planes[p, blk*126 + i*6 + g], 6 pixel-column groups per matmul
    ([128, 126] x [128, 126]) accumulated into one PSUM [126, 126] f32 tile
    (exact integer counts < 2^24),
  - tail padding uses value 0 -> pads land in TH[0,0] only; the host
    subtracts the deterministic pad count from bin [0, 0].

GpSimd is left COMPLETELY idle: its SBUF port is shared with VectorE and
any sustained GpSimd traffic destroys the DVE 4x (two-port) perf mode
(measured 8x DVE slowdown + 16us/op GpSimd is_equal).
"""

import numpy as np

C = 21                  # classes / thermometer levels
G = 6                   # pixel-column groups per matmul (G*C = 126 <= 128)
M = G * C               # 126 (moving width = PE cycles per matmul)
SW = 128                # stationary width: 128 cols enables FWL
MP = 130                # block pitch: non-power-of-2 stride avoids SBUF
                        # bank conflicts on the DVE's strided mask writes
P = 128                 # partitions
FP = 1008               # plane-chunk columns per tensor (divisible by 6)
MASK_DT = "bf16"

_CACHE = {}


def _build(
    n_free,
    work_cols=None,
    repeat=1,
    skip_mm=False,
    n_cls=C,
    mask_dt=MASK_DT,
):
    import concourse.bacc as bacc
    import concourse.mybir as mybir
    import concourse.tile as tile
    from contextlib import nullcontext

    if work_cols is None:
        work_cols = n_free

    nc = bacc.Bacc(
        "TRN2",
        target_bir_lowering=False,
        debug=False,
        enable_asserts=False,
        num_devices=8,
    )
    yp = nc.dram_tensor("yp", [P, n_free], mybir.dt.int32, kind="ExternalInput").ap()
    y = nc.dram_tensor("y", [P, n_free], mybir.dt.int32, kind="ExternalInput").ap()
    out = nc.dram_tensor("out", [M, M], mybir.dt.float32, kind="ExternalOutput").ap()

    # Graduated chunk widths: small first chunks cut the pipeline-fill
    # latency (PE can start after ~100 cols instead of a full FP chunk) and
    # small last chunks shorten the close-out chain after the final masks;
    # the remainder becomes a small padded tail chunk.
    ramp = []
    remaining = work_cols
    for wsmall in (96, 252, 504):
        if remaining >= wsmall + FP:
            ramp.append(wsmall)
            remaining -= wsmall
    rest = [FP] * (remaining // FP)
    remaining -= (remaining // FP) * FP
    tail_cols = remaining                            # < FP; single closeout
                                                     # chunk (fewer per-op
                                                     # fixed costs than
                                                     # 504+tail split)
    tail_pad = -tail_cols % G
    tail_w = tail_cols + tail_pad
    # (width, plane-buffer index, block offset): the ramp chunks share
    # buffer 0 at successive block offsets so the vector engines can mask
    # them back-to-back without waiting on PE buffer turnaround.
    chunk_plan = []
    blk = 0
    for w in ramp:
        chunk_plan.append((w, 0, blk))
        blk += 2 * w // G
    assert blk <= 2 * FP // G
    for j, w in enumerate(rest):
        chunk_plan.append((w, (1 + j) % 2, 0))
    tail_buf = (1 + len(rest)) % 2
    widths = ramp + rest
    total_mms = sum(w // G for w in widths) + (tail_w // G)

    mdt = {"bf16": mybir.dt.bfloat16, "fp8": mybir.dt.float8e4}[mask_dt]
    bf16 = mybir.dt.bfloat16
    f32 = mybir.dt.float32
    i32 = mybir.dt.int32
    Copy = mybir.ActivationFunctionType.Copy
    Relu = mybir.ActivationFunctionType.Relu
    Sign = mybir.ActivationFunctionType.Sign
    Sigmoid = mybir.ActivationFunctionType.Sigmoid
    dve_lvls = list(range(2, n_cls - 3))             # 2..17 on VectorE

    with tile.TileContext(nc) as tc:
        with (
            tc.tile_pool(name="psum", bufs=1, space="PSUM") as psum_pool,
            tc.tile_pool(name="cat", bufs=2) as cat_pool,
            tc.tile_pool(name="singles", bufs=1) as singles,
        ):
            acc = psum_pool.tile([P, M], f32)
            # Preload both ACT spline table sets (Copy/Sign/Relu set and
            # Sigmoid set) with dummy 1-col activations so the ~1.3us
            # ACT_TABLE_LOADs overlap the initial DMA instead of sitting on
            # the first chunk's critical path.
            warm = singles.tile([P, 1], f32, tag="actwarm")
            nc.scalar.activation(warm[:], warm[:], Copy)
            nc.scalar.activation(warm[:], warm[:], Sigmoid)
            sig_bias = {}
            for i in (2, n_cls - 3, n_cls - 2):     # levels 2 (alt), 18, 19
                b = singles.tile([P, 1], f32, tag=f"sigb{i}")
                nc.vector.memset(b[:], -100.0 * (i - 0.5))
                sig_bias[i] = b
            bias_ramp = singles.tile([P, 1], f32, tag="biasr")
            nc.vector.memset(bias_ramp[:], -(2.0 * (n_cls - 1) - 1.0))

            # Two manually ping-ponged plane buffers; their level-0 slice is
            # an all-ones constant written once and never touched again.
            # Block pitch is MP=128: cols 126-127 of each block are padding
            # (zeroed once) so the stationary operand is a full 128-column
            # weight load, which enables fast-weight-load on the PE.
            # (block cols M..MP-1 stay uninitialized: as stationary weights
            # they only feed PSUM rows 126-127, which are never read)
            plane_bufs = []
            for pb in range(2):
                pl = singles.tile([P, (2 * FP // G) * MP], mdt,
                                  tag=f"planes{pb}")
                plane_bufs.append(pl)

            def _ones_memset(pb):
                pl3f = plane_bufs[pb][:].rearrange("p (b f) -> p b f", f=MP)
                nc.vector.memset(pl3f[:, :, 0:G], 1.0)

            _ones_memset(0)

            # HAM warm-up: the PE clock-gate starts at 1.2 GHz and needs
            # ~3.4us of sustained matmul activity to reach 2.4 GHz. Run
            # dummy matmuls on plane-buffer blocks the ramp never touches
            # (blocks 300+ of buf 0) while the first DMA+masks are pending,
            # so the real ramp matmuls start warm (55ns instead of 106ns).
            mm = 0
            rep_ctx = tc.For_i(0, repeat, 1) if repeat > 1 else nullcontext()

            with rep_ctx:

                def do_plane_chunk(cat32, w, planes, blk0, alt=False):
                    """cat32: [128, 2*w] int32 = [yp vals | y vals], w % 6 == 0.

                    planes[p, blk*MP + i*6 + g] = (vals[p, blk*6+g] >= i),
                    blk in [0, 2*w/6). A-side = blks [0, w/6), B-side = rest.
                    Each matmul reads a contiguous slice.

                    alt=True moves level 2 from DVE to ScalarE (saturated
                    sigmoid); alternating per chunk balances the two engines
                    (DVE 16 ops vs ScalarE 5 is DVE-heavy, 15 vs 6 is
                    ScalarE-heavy -- the average is the wall).
                    """
                    nonlocal mm
                    nblk = 2 * w // G
                    cat16 = cat_pool.tile([P, 2 * FP], bf16, tag="cat16")
                    c16 = cat16[:, : 2 * w]
                    nc.scalar.activation(c16[:], cat32[:], Copy)
                    pl3 = planes[:, blk0 * MP : (blk0 + nblk) * MP].rearrange(
                        "p (b f) -> p b f", f=MP
                    )
                    cat3 = c16[:].rearrange("p (b f) -> p b f", f=G)
                    cat3_32 = cat32[:].rearrange("p (b f) -> p b f", f=G)
                    sig_lvls = [2, n_cls - 3, n_cls - 2] if alt else \
                        [n_cls - 3, n_cls - 2]
                    for i in (dve_lvls[1:] if alt else dve_lvls):
                        nc.vector.tensor_scalar(
                            pl3[:, :, i * G : (i + 1) * G],
                            cat3[:],
                            float(i),
                            None,
                            mybir.AluOpType.is_ge,
                        )
                    # level 1: Sign(x) = (x >= 1) for x in {0..20}
                    nc.scalar.activation(
                        pl3[:, :, 1 * G : 2 * G], cat3_32[:], Sign, bias=0.0
                    )
                    # levels 2 (alt), 18, 19: saturated sigmoid step at i - 0.5
                    for i in sig_lvls:
                        nc.scalar.activation(
                            pl3[:, :, i * G : (i + 1) * G],
                            cat3_32[:],
                            Sigmoid,
                            bias=sig_bias[i][:],
                            scale=100.0,
                        )
                    # level 20: relu(2x - 39) = (x >= 20) for x in {0..20}
                    i = n_cls - 1
                    nc.scalar.activation(
                        pl3[:, :, i * G : (i + 1) * G],
                        cat3_32[:],
                        Relu,
                        bias=bias_ramp[:],
                        scale=2.0,
                    )
                    a0 = blk0 * MP
                    b0 = (blk0 + w // G) * MP
                    for t in (range(0) if skip_mm else range(w // G)):
                        # stationary: full 128-col block (2 pad cols -> PSUM
                        # rows 126-127, ignored); moving: 126 cols only.
                        nc.tensor.matmul(
                            acc[:, :],
                            planes[:, a0 + t * MP : a0 + t * MP + SW],
                            planes[:, b0 + t * MP : b0 + t * MP + M],
                            start=(mm == 0),
                            stop=(mm == total_mms - 1),
                        )
                        mm += 1

                off = 0
                n_alt = 0
                for ci, (w, bidx, blk0) in enumerate(chunk_plan):
                    ct = cat_pool.tile([P, 2 * FP], i32, tag="cat32")
                    ctw = ct[:, : 2 * w]
                    nc.sync.dma_start(ctw[:, :w], yp[:, off : off + w])
                    nc.sync.dma_start(ctw[:, w:], y[:, off : off + w])
                    do_plane_chunk(ctw, w, plane_bufs[bidx], blk0)
                    off += w
                    if ci == 1:
                        # buf 1's all-ones plane: emitted here so it slots
                        # into the ramp instead of delaying chunk 0's masks;
                        # first needed by the chunk-3 matmuls.
                        _ones_memset(1)

                if tail_cols:
                    ct = cat_pool.tile([P, 2 * FP], i32, tag="cat32")
                    ctw = ct[:, : 2 * tail_w]
                    if tail_pad:
                        # pad value 0: pad pixels land in TH[0,0] only; the
                        # host subtracts the deterministic pad count.
                        nc.vector.memset(ctw[:], 0)
                    nc.sync.dma_start(
                        ctw[:, :tail_cols], yp[:, off : off + tail_cols]
                    )
                    nc.sync.dma_start(
                        ctw[:, tail_w : tail_w + tail_cols],
                        y[:, off : off + tail_cols],
                    )
                    do_plane_chunk(ctw, tail_w, plane_bufs[tail_buf], 0)

            assert skip_mm or mm == total_mms
            res = singles.tile([M, M], f32)
            if skip_mm:
                nc.vector.memset(res[:], 0.0)
            else:
                nc.vector.tensor_copy(res[:], acc[0:M, :])
            nc.sync.dma_start(out, res[:])

    nc.compile()
    n_pad_px = tail_pad * P
    return nc, n_pad_px


def _get(n_free):
    if n_free not in _CACHE:
        _CACHE[n_free] = _build(n_free)
    return _CACHE[n_free]


def kernel(yp, y, res, n_classes, _trace=False):
    from concourse import bass_utils

    yp = np.ascontiguousarray(np.asarray(yp))
    y = np.ascontiguousarray(np.asarray(y))
    B = yp.shape[0]
    n_free = yp[0].size // P
    nc, n_pad_px = _get(n_free)
    in_maps = [
        {"yp": yp[b].reshape(P, n_free), "y": y[b].reshape(P, n_free)}
        for b in range(B)
    ]
    r = bass_utils.run_bass_kernel_spmd(
        nc, in_maps, core_ids=list(range(B)), trace=_trace
    )
    outs = []
    for b in range(B):
        Pm = r.results[b]["out"].astype(np.float64)
        TH = np.zeros((C, C), np.float64)
        for g in range(G):
            TH += Pm[g::G, g::G]
        TH[0, 0] -= n_pad_px
        # counts = 2-D finite difference of the cumulative (>=, >=) matrix
        THp = np.zeros((C + 1, C + 1), np.float64)
        THp[:C, :C] = TH
        Rb = THp[:C, :C] - THp[1:, :C] - THp[:C, 1:] + THp[1:, 1:]
        outs.append(Rb)
    res_np = np.stack(outs).astype(np.float32)
    if _trace:
        kernel._last_results = r
    return res_np

